# revision 1
# baseline (speedup 1.0000x reference)
"""Trainium2 Bass kernel for nn_CrossAttentionMatrix.

Math (per batch b):
    m[c]  = sum_s y[b, c, s]                      (s over h*w = 65536)
    G[b,s] = (sum_c x[b, c, s] * m[c]) / (hw * hw * c)
Output: G reshaped (n, h, w).

Sharding: data-parallel over batch n=16 across 8 cores (2 batches/core).

Per-core layout (all fp32):
  - x[b], y[b] are (64, 65536) row-major = flat 4194304 elements, viewed as
    (128, 32768): partition p <-> channel p//2, spatial half p%2.
  - y reduction: chunked free-dim reduce_sum on VectorE -> ysum_f (128, 1).
  - weight build: pairsum[i] = ysum_f[2*(i//2)] + ysum_f[2*(i//2)+1] via a
    tiny matmul with a constant block-diagonal matrix, then W = pairsum *
    mask, mask[i, j] = S * (i%2 == j).  W is (128, 2).
  - x matvec: matmul(lhsT=W (128,2), rhs=x_tile (128,512)) -> psum (2, 512):
    row h = G[h*32768 + 512*t : ...].  Run in float32r (1 col/cycle).
  - evacuate psum on ScalarE into a (2, 16384) staging tile, DMA to DRAM.
"""

import numpy as np

N_CORES = 8
B_PER_CORE = 2
C = 64
H = 256
W = 256
HW = H * W                    # 65536
P = 128                       # SBUF partitions
FREE = C * HW // P            # 32768 free elems per partition per batch
CH = 4096                     # DMA chunk (128, 4096) = 2 MiB
NCH = FREE // CH              # 8 chunks per batch
MMN = 512                     # matmul moving free dim (one PSUM bank)
MM_PER_CH = CH // MMN         # 8
OUT_STAGE = 4096              # staging free elems (2, 4096) = 8 matmuls
SCALE = 1.0 / (float(HW) * float(HW) * float(C))

_NC_CACHE = {}


def _build_nc():
    import concourse.bacc as bacc
    import concourse.tile as tile
    from concourse import mybir

    f32 = mybir.dt.float32
    f32r = mybir.dt.float32r
    AX = mybir.AxisListType

    nc = bacc.Bacc("TRN2", target_bir_lowering=False)

    x_d = nc.dram_tensor("x", (B_PER_CORE, P, FREE), f32r, kind="ExternalInput")
    y_d = nc.dram_tensor("y", (B_PER_CORE, P, FREE), f32, kind="ExternalInput")
    out_d = nc.dram_tensor("out", (B_PER_CORE, 2, HW // 2), f32, kind="ExternalOutput")

    # Constants.
    pp = np.zeros((P, P), np.float32)          # block-diag pair-sum matrix
    for i in range(P):
        base = (i // 2) * 2
        pp[base, i] = 1.0
        pp[base + 1, i] = 1.0
    mask = np.zeros((P, 2), np.float32)
    for i in range(P):
        mask[i, i % 2] = SCALE
    pp_d = nc.inline_tensor(pp, name="pp_const")
    mask_d = nc.inline_tensor(mask, name="mask_const")

    PSN = 512
    MM_PER_PS = 1

    with tile.TileContext(nc) as tc:
        with (
            tc.tile_pool(name="consts", bufs=1) as consts,
            tc.tile_pool(name="xpool", bufs=5) as xpool,
            tc.tile_pool(name="ypool", bufs=3) as ypool,
            tc.tile_pool(name="stats", bufs=2) as stats,
            tc.tile_pool(name="small", bufs=2) as small,
            tc.tile_pool(name="pswp", bufs=1, space="PSUM") as pswp,
            tc.tile_pool(name="mmp", bufs=7, space="PSUM") as mmp,
            tc.tile_pool(name="outp", bufs=3) as outp,
        ):
            pp_sb = consts.tile([P, P], f32)
            nc.gpsimd.dma_start(out=pp_sb, in_=pp_d[:, :])
            mask_sb = consts.tile([P, 2], f32)
            nc.gpsimd.dma_start(out=mask_sb, in_=mask_d[:, :])

            def emit_y_chunk(b, k, ysum_parts):
                yt = ypool.tile([P, CH], f32)
                nc.gpsimd.dma_start(out=yt, in_=y_d[b, :, k * CH:(k + 1) * CH])
                nc.vector.reduce_sum(
                    out=ysum_parts[:, k:k + 1], in_=yt, axis=AX.X
                )

            def emit_w_build(ysum_parts):
                ysum2 = small.tile([P, 2], f32, tag="ysum2")
                nc.vector.reduce_sum(out=ysum2[:, 0:1], in_=ysum_parts, axis=AX.X)
                nc.vector.tensor_copy(out=ysum2[:, 1:2], in_=ysum2[:, 0:1])
                psw = pswp.tile([P, 2], f32)
                nc.tensor.matmul(psw, lhsT=pp_sb[:, :], rhs=ysum2[:, :],
                                 start=True, stop=True)
                w_sb = small.tile([P, 2], f32r, tag="w")
                nc.vector.tensor_mul(w_sb, psw, mask_sb)
                return w_sb

            def emit_x_chunk(b, k, w_sb):
                xt = xpool.tile([P, CH], f32r)
                nc.gpsimd.dma_start(out=xt, in_=x_d[b, :, k * CH:(k + 1) * CH])
                out_sb = outp.tile([2, OUT_STAGE], f32)
                ps = None
                for n in range(MM_PER_CH):
                    t = k * MM_PER_CH + n
                    if n % MM_PER_PS == 0:
                        ps = mmp.tile([2, PSN], f32)
                    j = n % MM_PER_PS
                    nc.tensor.matmul(
                        ps[:, j * MMN:(j + 1) * MMN],
                        lhsT=w_sb[:, :],
                        rhs=xt[:, n * MMN:(n + 1) * MMN],
                        start=True, stop=True,
                    )
                    if (n + 1) % MM_PER_PS == 0:
                        off = (n - MM_PER_PS + 1) * MMN
                        nc.scalar.copy(out=out_sb[:, off:off + PSN], in_=ps)
                nc.gpsimd.dma_start(
                    out=out_d[b, :, k * OUT_STAGE:(k + 1) * OUT_STAGE],
                    in_=out_sb,
                )

            # Software pipeline: batch b's x phase interleaved with batch
            # b+1's y phase so the single SWDGE trigger stream alternates
            # between the two and DMA stays saturated.
            ysp = {0: stats.tile([P, NCH], f32, tag="ysum_parts", name="ysp0")}
            for k in range(NCH):
                emit_y_chunk(0, k, ysp[0])
            wsb = {0: emit_w_build(ysp[0])}
            for b in range(B_PER_CORE):
                nb = b + 1
                if nb < B_PER_CORE:
                    ysp[nb] = stats.tile([P, NCH], f32, tag="ysum_parts", name=f"ysp{nb}")
                for k in range(NCH):
                    if nb < B_PER_CORE:
                        emit_y_chunk(nb, k, ysp[nb])
                    emit_x_chunk(b, k, wsb[b])
                if nb < B_PER_CORE:
                    wsb[nb] = emit_w_build(ysp[nb])
    nc.compile()
    return nc


def _get_nc():
    if "nc" not in _NC_CACHE:
        _NC_CACHE["nc"] = _build_nc()
    return _NC_CACHE["nc"]


def kernel(**inputs):
    x = np.ascontiguousarray(np.asarray(inputs["x"], dtype=np.float32))
    y = np.ascontiguousarray(np.asarray(inputs["y"], dtype=np.float32))
    n = x.shape[0]
    assert x.shape == (n, C, H, W) and n == N_CORES * B_PER_CORE

    from concourse import bass_utils

    nc = _get_nc()
    xs = x.reshape(N_CORES, B_PER_CORE, P, FREE)
    ys = y.reshape(N_CORES, B_PER_CORE, P, FREE)
    in_maps = [
        {"x": np.ascontiguousarray(xs[i]), "y": np.ascontiguousarray(ys[i])}
        for i in range(N_CORES)
    ]
    res = bass_utils.run_bass_kernel_spmd(nc, in_maps, core_ids=list(range(N_CORES)))
    outs = [r["out"].reshape(B_PER_CORE, H, W) for r in res.results]
    return np.concatenate(outs, axis=0)



# revision 2
# speedup vs baseline: 1.9824x; 1.9824x over previous
"""Trainium2 Bass kernel for nn_CrossAttentionMatrix.

Math (per batch b):
    m[c]   = sum_s y[b, c, s]                     (s over h*w = 65536)
    G[b,s] = (sum_c x[b, c, s] * m[c]) / (hw * hw * c)
Output: G reshaped (n, h, w).

Sharding: data-parallel over batch n=16 across 8 cores, 2 batches/core.
Both batches are stacked on the partition axis: partition p <-> (batch
p//64, channel p%64), so one weight vector drives both batches' matvecs
and each matmul emits 2 output rows (one per batch).

Inputs are cast to bf16 on the host (tolerance is 2e-2; bf16 keeps the
result near 3e-3) which halves HBM traffic -- the sole bottleneck.

Per-core structure (all DMA chunks are (128, 8192) bf16 = 2 MiB):
  y phase: 8 chunk loads on the sync-engine HWDGE ring; each chunk is
    free-dim reduce_summed on VectorE into ysum_parts (128, 8).
  w build: reduce ysum_parts -> ysum (128,1); ScalarE ACTIVATE computes
    w = mask * ysum (mask[p,q] = SCALE * (p//64 == q)), cast to bf16.
  x phase: 8 chunk loads on the same sync ring (prefetched during the w
    barrier).  Each chunk = 16 matmul blocks of 512 cols.  Blocks are
    packed 4-per-PSUM-bank via col-strip tile_position=(0,32n): block
    (c,n) writes psum rows 32n..32n+1 of bank-tile c, so one ScalarE
    copy evacuates 4 blocks at 8 active partitions -> staging
    (128, 2048).  Two HWDGE DMAs per chunk (one per batch, partition
    stride 32) store to DRAM, triggered from the scalar queue so the
    input ring never waits on compute.
"""

import numpy as np

N_CORES = 8
B_PER_CORE = 2
C = 64
H = 256
W = 256
HW = H * W                     # 65536
P = 128                        # SBUF partitions = B_PER_CORE * C
CH = 8192                      # chunk free elems (bf16) = 16 KiB/partition
NCH = HW // CH                 # 8 chunks
MMN = 512                      # matmul moving dim = one PSUM bank (f32)
NSTRIP = 4                     # col strips per PSUM bank-tile
NBANK = CH // (MMN * NSTRIP)   # bank-tiles per chunk = 4
SCALE = 1.0 / (float(HW) * float(HW) * float(C))   # exactly 2**-38

_NC_CACHE = {}


def _build_nc():
    import concourse.bacc as bacc
    import concourse.tile as tile
    from concourse import mybir

    f32 = mybir.dt.float32
    bf16 = mybir.dt.bfloat16
    AX = mybir.AxisListType

    nc = bacc.Bacc("TRN2", target_bir_lowering=False)

    x_d = nc.dram_tensor("x", (P, HW), bf16, kind="ExternalInput")
    y_d = nc.dram_tensor("y", (P, HW), bf16, kind="ExternalInput")
    # [q, k, c, n, j]: flat offset = q*65536 + k*8192 + c*2048 + n*512 + j
    # == q*65536 + spatial index of block (k, 4c+n) col j.
    out_d = nc.dram_tensor("out", (B_PER_CORE, NCH, NBANK, NSTRIP, MMN), f32,
                           kind="ExternalOutput")

    mask = np.zeros((P, B_PER_CORE), np.float32)
    for p in range(P):
        mask[p, p // C] = SCALE
    mask_d = nc.inline_tensor(mask, name="mask_const")

    with tile.TileContext(nc) as tc:
        with (
            tc.tile_pool(name="consts", bufs=1) as consts,
            tc.tile_pool(name="xpool", bufs=5) as xpool,
            tc.tile_pool(name="ypool", bufs=3) as ypool,
            tc.tile_pool(name="stats", bufs=1) as stats,
            tc.tile_pool(name="small", bufs=1) as small,
            tc.tile_pool(name="mmp", bufs=8, space="PSUM") as mmp,
            tc.tile_pool(name="outp", bufs=3) as outp,
        ):
            mask_sb = consts.tile([P, B_PER_CORE], f32)
            nc.gpsimd.dma_start(out=mask_sb, in_=mask_d[:, :])

            # ---- y phase: load + free-dim reduce per chunk ----
            ysum_parts = stats.tile([P, NCH], f32)
            for k in range(NCH):
                yt = ypool.tile([P, CH], bf16, tag="yt")
                nc.sync.dma_start(out=yt, in_=y_d[:, k * CH:(k + 1) * CH])
                nc.vector.reduce_sum(
                    out=ysum_parts[:, k:k + 1], in_=yt, axis=AX.X
                )

            # ---- w build ----
            ysum = small.tile([P, 1], f32, tag="ysum")
            nc.vector.reduce_sum(out=ysum, in_=ysum_parts, axis=AX.X)
            w_sb = small.tile([P, B_PER_CORE], bf16, tag="w")
            # w[p, q] = mask[p, q] * ysum[p]  (cast to bf16 on write)
            nc.scalar.activation(
                out=w_sb, in_=mask_sb,
                func=mybir.ActivationFunctionType.Copy, scale=ysum[:, 0:1],
            )

            # ---- x phase ----
            for k in range(NCH):
                xt = xpool.tile([P, CH], bf16, tag="xt")
                nc.sync.dma_start(out=xt, in_=x_d[:, k * CH:(k + 1) * CH])
                stage = outp.tile([P, NBANK * MMN], f32, tag="stage")
                for c in range(NBANK):
                    ps = mmp.tile([P, MMN], f32, tag="ps")
                    for n in range(NSTRIP):
                        b = c * NSTRIP + n
                        nc.tensor.matmul(
                            ps[32 * n:32 * n + B_PER_CORE, :],
                            lhsT=w_sb[:, :],
                            rhs=xt[:, b * MMN:(b + 1) * MMN],
                            start=True, stop=True,
                            tile_position=(0, 32 * n),
                        )
                    nc.scalar.copy(
                        out=stage[:, c * MMN:(c + 1) * MMN], in_=ps
                    )
                for q in range(B_PER_CORE):
                    nc.scalar.dma_start(
                        out=out_d[q, k].transpose([1, 0, 2]),
                        in_=stage[q:97 + q:32, :].rearrange(
                            "p (c j) -> p c j", j=MMN
                        ),
                    )
    nc.compile()
    return nc


def _get_nc():
    if "nc" not in _NC_CACHE:
        _NC_CACHE["nc"] = _build_nc()
    return _NC_CACHE["nc"]


def _prep_in_maps(x, y):
    import ml_dtypes

    bf16 = ml_dtypes.bfloat16
    n = x.shape[0]
    assert x.shape == (n, C, H, W) and n == N_CORES * B_PER_CORE
    xs = np.asarray(x, dtype=bf16).reshape(N_CORES, P, HW)
    ys = np.asarray(y, dtype=bf16).reshape(N_CORES, P, HW)
    return [
        {"x": np.ascontiguousarray(xs[i]), "y": np.ascontiguousarray(ys[i])}
        for i in range(N_CORES)
    ]


def _assemble(results):
    outs = [r["out"].reshape(B_PER_CORE, H, W) for r in results]
    return np.concatenate(outs, axis=0)


def kernel(**inputs):
    x = np.ascontiguousarray(np.asarray(inputs["x"], dtype=np.float32))
    y = np.ascontiguousarray(np.asarray(inputs["y"], dtype=np.float32))

    from concourse import bass_utils

    nc = _get_nc()
    in_maps = _prep_in_maps(x, y)
    res = bass_utils.run_bass_kernel_spmd(nc, in_maps, core_ids=list(range(N_CORES)))
    return _assemble(res.results)
